# revision 1
# baseline (speedup 1.0000x reference)
"""3-layer GCN (message passing) on 8 NeuronCores via Bass/Tile.

Strategy (vertex-cut / dst-sharding):
  - Node i's output row is owned by core i // 6250.
  - out = relu(A_hat @ (X W) + b) per layer, A_hat = D^-1/2 (A+I) D^-1/2.
  - Fold dinv[src] into the gather table (Y = dinv * (X W)); fold dinv[dst]
    into host-built one-hot scatter matrices S (S[e, dst_rel] = dinv[dst]).
  - Scatter-add realized as TensorE matmuls: psum[dst,feat] += S^T @ Y[src].
  - Layer 1 table computed redundantly per core from the replicated input X.
    Layers 2/3: each core computes its Y shard, AllGather to a full table.
"""

import os
import sys

sys.path.insert(0, "/opt/trn_rl_repo")

import numpy as np

N = 50000
E = 500000
NC = 8
SH = N // NC            # 6250 nodes per core
P = 128
DIN = 128
DH = 256
NBLK = (SH + P - 1) // P  # 49 dst blocks per core; last block has 106 nodes
LASTM = SH - (NBLK - 1) * P  # 106
NW = (N + P - 1) // P   # 391 windows over all nodes; last has 80
LASTW = N - (NW - 1) * P  # 80


def _preprocess(x, edge_index):
    """Host-side graph partitioning. Returns per-core tensors + layout meta."""
    src = np.asarray(edge_index[0], dtype=np.int64)
    dst = np.asarray(edge_index[1], dtype=np.int64)
    deg = np.bincount(dst, minlength=N).astype(np.float64) + 1.0
    dinv = (1.0 / np.sqrt(deg)).astype(np.float32)

    order = np.argsort(dst, kind="stable")
    s_s = src[order]
    d_s = dst[order]

    # per-core edge ranges (d_s sorted ascending)
    bounds = np.searchsorted(d_s, np.arange(NC + 1) * SH)

    # chunk counts per (core, block)
    cnts = np.zeros((NC, NBLK), dtype=np.int64)
    for c in range(NC):
        lo, hi = bounds[c], bounds[c + 1]
        blk = (d_s[lo:hi] - c * SH) >> 7
        cnts[c] = np.bincount(blk, minlength=NBLK)
    cpb = np.maximum(1, (cnts.max(axis=0) + P - 1) // P)  # chunks per block
    cum = np.concatenate([[0], np.cumsum(cpb)])           # edge-chunk offsets
    nch = int(cum[-1])
    # gather-column offsets: per block cp edge chunks + 1 self chunk
    gcum = cum[:-1] + np.arange(NBLK)
    ngc = nch + NBLK

    per_core = []
    for c in range(NC):
        lo, hi = bounds[c], bounds[c + 1]
        sc = s_s[lo:hi]
        dc = (d_s[lo:hi] - c * SH).astype(np.int64)
        blk = dc >> 7
        n_e = hi - lo
        starts = np.concatenate([[0], np.cumsum(cnts[c])])
        pos = np.arange(n_e) - starts[blk]
        chunk = cum[blk] + (pos >> 7)
        prow = pos & 127

        idx_all = np.zeros((P, ngc), dtype=np.int32)
        s_all = np.zeros((P, nch * P), dtype=np.float32)
        gcol = gcum[blk] + (pos >> 7)
        idx_all[prow, gcol] = sc.astype(np.int32)
        s_all[prow, chunk * P + (dc & 127)] = dinv[dc + c * SH]

        # self columns: block b -> gather column gcum[b] + cpb[b]
        ids = c * SH + np.arange(NBLK * P)
        valid = ids < (c + 1) * SH
        ids_c = np.where(valid, ids, 0).reshape(NBLK, P).T.astype(np.int32)
        selfcol = (gcum + cpb).astype(np.int64)
        idx_all[:, selfcol] = ids_c

        dcol = np.where(valid, dinv[np.minimum(ids, N - 1)], 0.0)
        dcol = dcol.reshape(NBLK, P).T.astype(np.float32)
        per_core.append({"idx": idx_all, "sall": s_all, "dcol": dcol})

    # dinv for all nodes, [P, NW] layout (window-major)
    dpad = np.zeros(NW * P, dtype=np.float32)
    dpad[:N] = dinv
    dfull = dpad.reshape(NW, P).T.copy()

    meta = {"cpb": cpb.tolist(), "cum": cum.tolist(), "gcum": gcum.tolist(),
            "nch": nch, "ngc": ngc}
    return per_core, dfull, meta


def _build_program(meta):
    from concourse import bass, bacc, mybir
    import concourse.tile as tile
    from concourse.masks import make_identity

    f32 = mybir.dt.float32
    i32 = mybir.dt.int32
    cpb, gcum, cum = meta["cpb"], meta["gcum"], meta["cum"]
    nch, ngc = meta["nch"], meta["ngc"]

    nc = bacc.Bacc("TRN2", target_bir_lowering=False, debug=False)

    xt = nc.declare_dram_parameter("xt", [P, N], f32, isOutput=False)
    w1 = nc.declare_dram_parameter("w1", [DIN, DH], f32, isOutput=False)
    w2 = nc.declare_dram_parameter("w2", [DH, DH], f32, isOutput=False)
    w3 = nc.declare_dram_parameter("w3", [DH, DH], f32, isOutput=False)
    bf1 = nc.declare_dram_parameter("bf1", [P, DH], f32, isOutput=False)
    bf2 = nc.declare_dram_parameter("bf2", [P, DH], f32, isOutput=False)
    bf3 = nc.declare_dram_parameter("bf3", [P, DH], f32, isOutput=False)
    idx = nc.declare_dram_parameter("idx", [P, ngc], i32, isOutput=False)
    sall = nc.declare_dram_parameter("sall", [P, nch * P], f32, isOutput=False)
    dcol = nc.declare_dram_parameter("dcol", [P, NBLK], f32, isOutput=False)
    dful = nc.declare_dram_parameter("dful", [P, NW], f32, isOutput=False)
    outp = nc.declare_dram_parameter("out", [SH, DH], f32, isOutput=True)

    # internal DRAM
    y1 = nc.dram_tensor("y1", [N, DH], f32)           # L1 gather table (full, local)
    ybin2 = nc.dram_tensor("ybin2", [SH, DH], f32)
    ybout2 = nc.dram_tensor("ybout2", [N, DH], f32, addr_space="Shared")
    ybin3 = nc.dram_tensor("ybin3", [SH, DH], f32)
    ybout3 = nc.dram_tensor("ybout3", [N, DH], f32, addr_space="Shared")

    WIN_PER_XBIG = 16

    with tile.TileContext(nc, linearize=bool(__import__("os").environ.get("KLIN"))) as tc:
        with (
            tc.tile_pool(name="const", bufs=1) as cp_,
            tc.tile_pool(name="sb", bufs=3) as sb,
            tc.tile_pool(name="gp", bufs=2) as gp,
            tc.tile_pool(name="xb", bufs=2) as xbp,
            tc.tile_pool(name="pp", bufs=2, space="PSUM") as pp,
        ):
            ident = cp_.tile([P, P], dtype=f32)
            make_identity(nc, ident[:])
            w1sb = cp_.tile([P, DH], dtype=f32)
            nc.sync.dma_start(out=w1sb[:], in_=w1[:, :])
            w2sb = cp_.tile([P, 2 * DH], dtype=f32)
            w3sb = cp_.tile([P, 2 * DH], dtype=f32)
            for k in range(2):
                nc.sync.dma_start(out=w2sb[:, k * DH:(k + 1) * DH],
                                  in_=w2[k * P:(k + 1) * P, :])
                nc.sync.dma_start(out=w3sb[:, k * DH:(k + 1) * DH],
                                  in_=w3[k * P:(k + 1) * P, :])
            bsb = []
            for bt in (bf1, bf2, bf3):
                t = cp_.tile([P, DH], dtype=f32, tag=f"b_{bt.name}")
                nc.sync.dma_start(out=t[:], in_=bt[:, :])
                bsb.append(t)
            idxsb = cp_.tile([P, ngc], dtype=i32)
            nc.sync.dma_start(out=idxsb[:], in_=idx[:, :])
            dcolsb = cp_.tile([P, NBLK], dtype=f32)
            nc.sync.dma_start(out=dcolsb[:], in_=dcol[:, :])
            dfulsb = cp_.tile([P, NW], dtype=f32)
            nc.sync.dma_start(out=dfulsb[:], in_=dful[:, :])
            # resident transposed activations for layers 2/3: [feat, 2*SH]
            xts = cp_.tile([P, 2 * SH], dtype=f32)

            AG = mybir.AluOpType
            ACT = mybir.ActivationFunctionType

            # ---------------- Layer 1 phase 1: full Y1 (redundant) ----------
            for t in range((NW + WIN_PER_XBIG - 1) // WIN_PER_XBIG):
                wlo = t * WIN_PER_XBIG
                whi = min(wlo + WIN_PER_XBIG, NW)
                ncols = min(whi * P, N) - wlo * P
                xbig = xbp.tile([P, WIN_PER_XBIG * P], dtype=f32, tag="xbig")
                nc.sync.dma_start(out=xbig[:, :ncols],
                                  in_=xt[:, wlo * P: wlo * P + ncols])
                for w in range(wlo, whi):
                    m = min(P, N - w * P)
                    ps = pp.tile([P, DH], dtype=f32, tag="hps")
                    nc.tensor.matmul(
                        out=ps[:m, :],
                        lhsT=xbig[:, (w - wlo) * P:(w - wlo) * P + m],
                        rhs=w1sb[:], start=True, stop=True)
                    ysb = sb.tile([P, DH], dtype=f32, tag="ysb")
                    nc.scalar.activation(out=ysb[:m, :], in_=ps[:m, :],
                                         func=ACT.Copy,
                                         scale=dfulsb[:m, w:w + 1])
                    nc.sync.dma_start(out=y1[w * P: w * P + m, :],
                                      in_=ysb[:m, :])

            def scatter_layer(l, table, b_tile, next_phase):
                """Scatter phase of layer l reading from full table."""
                for b in range(NBLK):
                    cp = cpb[b]
                    goff = gcum[b]
                    m = LASTM if b == NBLK - 1 else P
                    gt = gp.tile([P, (max(cpb) + 1) * DH], dtype=f32, tag="gt")
                    # HW DGE honors ONE index per partition per indirect DMA
                    # (extra offset columns are ignored; payload is read
                    # contiguously) -> one gather per 128-edge chunk.
                    for k in range(cp + 1):
                        nc.gpsimd.indirect_dma_start(
                            out=gt[:, k * DH:(k + 1) * DH], out_offset=None,
                            in_=table[:, :],
                            in_offset=bass.IndirectOffsetOnAxis(
                                ap=idxsb[:, goff + k:goff + k + 1], axis=0))
                    st = sb.tile([P, max(cpb) * P], dtype=f32, tag="st")
                    nc.sync.dma_start(
                        out=st[:, :cp * P],
                        in_=sall[:, cum[b] * P:(cum[b] + cp) * P])
                    ps = pp.tile([P, DH], dtype=f32, tag="agg")
                    for k in range(cp):
                        nc.tensor.matmul(out=ps[:], lhsT=st[:, k * P:(k + 1) * P],
                                         rhs=gt[:, k * DH:(k + 1) * DH],
                                         start=(k == 0), stop=(k == cp - 1))
                    # epilogue: out = [relu](psum + dinv*Y_self + b)
                    tmp = sb.tile([P, DH], dtype=f32, tag="tmp")
                    nc.vector.tensor_tensor(
                        out=tmp[:], in0=gt[:, cp * DH:(cp + 1) * DH],
                        in1=dcolsb[:, b:b + 1].to_broadcast([P, DH]),
                        op=AG.mult)
                    nc.vector.tensor_tensor(out=tmp[:], in0=tmp[:],
                                            in1=b_tile[:], op=AG.add)
                    if l < 3:
                        tmp2 = sb.tile([P, DH], dtype=f32, tag="tmp2")
                        nc.vector.tensor_tensor(out=tmp2[:], in0=tmp[:],
                                                in1=ps[:], op=AG.add)
                        xn = sb.tile([P, DH], dtype=f32, tag="xn")
                        nc.scalar.activation(out=xn[:], in_=tmp2[:],
                                             func=ACT.Relu)
                        for k in range(2):
                            tps = pp.tile([P, P], dtype=f32, tag="tp")
                            nc.tensor.transpose(
                                out=tps[:, :m],
                                in_=xn[:m, k * P:(k + 1) * P],
                                identity=ident[:m, :m])
                            nc.vector.tensor_copy(
                                out=xts[:, k * SH + b * P: k * SH + b * P + m],
                                in_=tps[:, :m])
                    else:
                        osb = sb.tile([P, DH], dtype=f32, tag="osb")
                        nc.vector.tensor_tensor(out=osb[:], in0=tmp[:],
                                                in1=ps[:], op=AG.add)
                        nc.sync.dma_start(out=outp[b * P: b * P + m, :],
                                          in_=osb[:m, :])
                if next_phase is not None:
                    next_phase()

            def phase1(wsb, ybin, ybout):
                """H = X@W for own shard from xts; write Y shard; AllGather."""
                for w in range(NBLK):
                    m = LASTM if w == NBLK - 1 else P
                    ps = pp.tile([P, DH], dtype=f32, tag="hps")
                    for k in range(2):
                        nc.tensor.matmul(
                            out=ps[:m, :],
                            lhsT=xts[:, k * SH + w * P: k * SH + w * P + m],
                            rhs=wsb[:, k * DH:(k + 1) * DH],
                            start=(k == 0), stop=(k == 1))
                    ysb = sb.tile([P, DH], dtype=f32, tag="ysb")
                    nc.scalar.activation(out=ysb[:m, :], in_=ps[:m, :],
                                         func=ACT.Copy,
                                         scale=dcolsb[:m, w:w + 1])
                    nc.sync.dma_start(out=ybin[w * P: w * P + m, :],
                                      in_=ysb[:m, :])
                nc.gpsimd.collective_compute(
                    "AllGather", AG.bypass,
                    replica_groups=[list(range(NC))],
                    ins=[ybin.ap().opt()],
                    outs=[ybout.ap().opt()])

            scatter_layer(1, y1, bsb[0],
                          lambda: phase1(w2sb, ybin2, ybout2))
            scatter_layer(2, ybout2, bsb[1],
                          lambda: phase1(w3sb, ybin3, ybout3))
            scatter_layer(3, ybout3, bsb[2], None)

            dbg = os.environ.get("KDBG")
            if dbg:
                src_t = {"y1": y1, "yb2": ybout2, "ybin2": ybin2,
                         "yb3": ybout3}[dbg]
                nc.sync.dma_start(out=outp[:, :], in_=src_t[0:SH, :])

    nc.compile()
    return nc


_CACHED = None


def _get_program_and_data(x, edge_index):
    global _CACHED
    per_core, dfull, meta = _preprocess(x, edge_index)
    nc = _build_program(meta)
    return nc, per_core, dfull


def kernel(x, edge_index, W1, b1, W2, b2, W3, b3, _trace=False):
    from concourse.bass_utils import run_bass_kernel_spmd

    x = np.asarray(x, dtype=np.float32)
    nc, per_core, dfull = _get_program_and_data(x, edge_index)

    xt = np.ascontiguousarray(x.T)
    common = {
        "xt": xt,
        "w1": np.asarray(W1, dtype=np.float32),
        "w2": np.asarray(W2, dtype=np.float32),
        "w3": np.asarray(W3, dtype=np.float32),
        "bf1": np.broadcast_to(np.asarray(b1, np.float32), (P, DH)).copy(),
        "bf2": np.broadcast_to(np.asarray(b2, np.float32), (P, DH)).copy(),
        "bf3": np.broadcast_to(np.asarray(b3, np.float32), (P, DH)).copy(),
        "dful": dfull,
    }
    in_maps = []
    for c in range(NC):
        m = dict(common)
        m["idx"] = per_core[c]["idx"]
        m["sall"] = per_core[c]["sall"]
        m["dcol"] = per_core[c]["dcol"]
        in_maps.append(m)

    res = run_bass_kernel_spmd(nc, in_maps, list(range(NC)), trace=_trace)
    shards = [res.results[c]["out"] for c in range(NC)]
    out = np.concatenate(shards, axis=0)
    if _trace:
        return out, res
    return out



# revision 9
# speedup vs baseline: 1.6191x; 1.6191x over previous
"""3-layer GCN (message passing) on 8 NeuronCores via Bass/Tile.

Strategy (vertex-cut / dst-sharding, bf16 data path):
  - Node i's output row is owned by core i // 6250; per-core edges grouped
    into 49 dst-blocks of 128, chunks of 128 edges (self-loops included as
    ordinary edges).
  - Layer 1 is re-associated: relu(A_hat (X W1) + b1) = relu((A_hat X') W1
    + b1) with X' = dinv*X pre-scaled on host and expanded to edge order
    (xe), so L1 needs no indirect gathers at all - pure streaming.
  - Layers 2/3: each core computes its Y = dinv*(h @ W) shard, AllGather
    (bf16) to a full table, then per-chunk indirect gathers (512B rows).
  - Scatter-add realized on TensorE: S[e, dst] built on-chip in one DVE op
    per chunk: S = (J == dstcol) * dval, dval = dinv[dst].
  - L1/L2 scatter runs transposed (psT = G^T @ S) so the ReLU epilogue
    writes h^T directly into the resident xts tile (no transposes);
    bias+relu fused into one ScalarE activation per feature half.
  - L3 scatter runs direct (ps = S^T @ G) to emit [dst, feat] fp32 rows.
"""

import os
import sys

sys.path.insert(0, "/opt/trn_rl_repo")

import numpy as np
import ml_dtypes

BF16 = ml_dtypes.bfloat16

N = 50000
E = 500000
NC = 8
SH = N // NC            # 6250 nodes per core
P = 128
DIN = 128
DH = 256
NBLK = (SH + P - 1) // P      # 49 dst blocks per core
LASTM = SH - (NBLK - 1) * P   # 106 dsts in the last block


def _preprocess(x, edge_index):
    """Host-side graph partitioning. Returns per-core tensors + layout meta."""
    src = np.asarray(edge_index[0], dtype=np.int64)
    dst = np.asarray(edge_index[1], dtype=np.int64)
    deg = np.bincount(dst, minlength=N).astype(np.float64) + 1.0
    dinv = (1.0 / np.sqrt(deg)).astype(np.float32)

    # append self loops as ordinary edges
    loop = np.arange(N, dtype=np.int64)
    src = np.concatenate([src, loop])
    dst = np.concatenate([dst, loop])

    order = np.argsort(dst, kind="stable")
    s_s = src[order]
    d_s = dst[order]
    bounds = np.searchsorted(d_s, np.arange(NC + 1) * SH)

    # chunk counts per (core, block); cpb shared across cores (SPMD program)
    cnts = np.zeros((NC, NBLK), dtype=np.int64)
    for c in range(NC):
        lo, hi = bounds[c], bounds[c + 1]
        blk = (d_s[lo:hi] - c * SH) >> 7
        cnts[c] = np.bincount(blk, minlength=NBLK)
    cpb = np.maximum(1, (cnts.max(axis=0) + P - 1) // P)
    cum = np.concatenate([[0], np.cumsum(cpb)])
    nch = int(cum[-1])

    xs = (dinv[:, None] * np.asarray(x, np.float32)).astype(BF16)  # [N, 128]

    per_core = []
    for c in range(NC):
        lo, hi = bounds[c], bounds[c + 1]
        sc = s_s[lo:hi]
        dc = d_s[lo:hi] - c * SH
        blk = dc >> 7
        n_e = hi - lo
        starts = np.concatenate([[0], np.cumsum(cnts[c])])
        pos = np.arange(n_e) - starts[blk]
        col = cum[blk] + (pos >> 7)     # chunk column
        prow = pos & 127                # partition (edge slot)

        idx_all = np.zeros((P, nch), dtype=np.int32)
        dcol_all = np.zeros((P, nch), dtype=np.float32)
        dval_all = np.zeros((P, nch), dtype=np.float32)
        idx_all[prow, col] = sc.astype(np.int32)
        dcol_all[prow, col] = (dc & 127).astype(np.float32)
        dval_all[prow, col] = dinv[dc + c * SH]

        # L1 expanded edge table: xe[p, col*128 + j] = xs[src, j] (0 pads)
        xe = np.zeros((P, nch, DIN), dtype=BF16)
        xe[prow, col, :] = xs[sc]
        xe = xe.reshape(P, nch * DIN)

        # dinv of own shard in [p, w] window layout
        ids = c * SH + np.arange(NBLK * P)
        valid = ids < (c + 1) * SH
        dc_own = np.where(valid, dinv[np.minimum(ids, N - 1)], 0.0)
        dc_own = dc_own.reshape(NBLK, P).T.astype(np.float32).copy()

        per_core.append({
            "idx": idx_all,
            "dcl": dcol_all.astype(BF16),
            "dvl": dval_all.astype(BF16),
            "xe": xe,
            "dco": dc_own,
        })

    meta = {"cpb": cpb.tolist(), "cum": cum.tolist(), "nch": nch}
    return per_core, meta


def _build_program(meta):
    from concourse import bass, bacc, mybir
    import concourse.tile as tile

    f32 = mybir.dt.float32
    bf16 = mybir.dt.bfloat16
    i32 = mybir.dt.int32
    cpb, cum, nch = meta["cpb"], meta["cum"], meta["nch"]
    mxcp = max(cpb)

    nc = bacc.Bacc("TRN2", target_bir_lowering=False, debug=False)

    xe = nc.declare_dram_parameter("xe", [P, nch * DIN], bf16, isOutput=False)
    idx = nc.declare_dram_parameter("idx", [P, nch], i32, isOutput=False)
    dcl = nc.declare_dram_parameter("dcl", [P, nch], bf16, isOutput=False)
    dvl = nc.declare_dram_parameter("dvl", [P, nch], bf16, isOutput=False)
    dco = nc.declare_dram_parameter("dco", [P, NBLK], f32, isOutput=False)
    w1 = nc.declare_dram_parameter("w1", [P, DH], bf16, isOutput=False)
    w2p = nc.declare_dram_parameter("w2p", [P, 2 * DH], bf16, isOutput=False)
    w3p = nc.declare_dram_parameter("w3p", [P, 2 * DH], bf16, isOutput=False)
    jt = nc.declare_dram_parameter("jt", [P, P], bf16, isOutput=False)
    bt = nc.declare_dram_parameter("bt", [P, 4], f32, isOutput=False)
    bf3 = nc.declare_dram_parameter("bf3", [P, DH], f32, isOutput=False)
    outp = nc.declare_dram_parameter("out", [SH, DH], f32, isOutput=True)

    ybin2 = nc.dram_tensor("ybin2", [SH, DH], bf16)
    ybout2 = nc.dram_tensor("ybout2", [N, DH], bf16, addr_space="Shared")
    ybin3 = nc.dram_tensor("ybin3", [SH, DH], bf16)
    ybout3 = nc.dram_tensor("ybout3", [N, DH], bf16, addr_space="Shared")

    AG = mybir.AluOpType
    ACT = mybir.ActivationFunctionType

    with tile.TileContext(nc, linearize=bool(os.environ.get("KLIN"))) as tc:
        with (
            tc.tile_pool(name="const", bufs=1) as cp_,
            tc.tile_pool(name="sb", bufs=3) as sb,
            tc.tile_pool(name="sp", bufs=4) as sp,
            tc.tile_pool(name="gp", bufs=3) as gp,
            tc.tile_pool(name="xb", bufs=2) as xbp,
            tc.tile_pool(name="pp", bufs=2, space="PSUM") as pp,
            tc.tile_pool(name="ph", bufs=6, space="PSUM") as ph,
        ):
            w1sb = cp_.tile([P, DH], dtype=bf16)
            nc.sync.dma_start(out=w1sb[:], in_=w1[:, :])
            w2sb = cp_.tile([P, 2 * DH], dtype=bf16)
            nc.sync.dma_start(out=w2sb[:], in_=w2p[:, :])
            w3sb = cp_.tile([P, 2 * DH], dtype=bf16)
            nc.sync.dma_start(out=w3sb[:], in_=w3p[:, :])
            jsb = cp_.tile([P, P], dtype=bf16)
            nc.sync.dma_start(out=jsb[:], in_=jt[:, :])
            btsb = cp_.tile([P, 4], dtype=f32)
            nc.sync.dma_start(out=btsb[:], in_=bt[:, :])
            bf3sb = cp_.tile([P, DH], dtype=f32)
            nc.sync.dma_start(out=bf3sb[:], in_=bf3[:, :])
            idxsb = cp_.tile([P, nch], dtype=i32)
            nc.sync.dma_start(out=idxsb[:], in_=idx[:, :])
            dclsb = cp_.tile([P, nch], dtype=bf16)
            nc.sync.dma_start(out=dclsb[:], in_=dcl[:, :])
            dvlsb = cp_.tile([P, nch], dtype=bf16)
            nc.sync.dma_start(out=dvlsb[:], in_=dvl[:, :])
            dcosb = cp_.tile([P, NBLK], dtype=f32)
            nc.sync.dma_start(out=dcosb[:], in_=dco[:, :])
            # resident transposed activations h^T: half h at cols [h*SH, ...)
            xts = cp_.tile([P, 2 * SH], dtype=bf16)

            def mk_s(colidx):
                """S[e, d] = (J[e, :] == dstcol[e]) * dval[e], one DVE op."""
                s = sp.tile([P, P], dtype=bf16, tag="s")
                nc.vector.scalar_tensor_tensor(
                    out=s[:],
                    in0=jsb[:],
                    scalar=dclsb[:, colidx:colidx + 1],
                    in1=dvlsb[:, colidx:colidx + 1].to_broadcast([P, P]),
                    op0=AG.is_equal, op1=AG.mult)
                return s

            # ---------------- Layer 1: streamed edge table ------------------
            for b in range(NBLK):
                cp = cpb[b]
                m = LASTM if b == NBLK - 1 else P
                xet = xbp.tile([P, mxcp * DIN], dtype=bf16, tag="xet")
                nc.sync.dma_start(
                    out=xet[:, :cp * DIN],
                    in_=xe[:, cum[b] * DIN:(cum[b] + cp) * DIN])
                psa = ph.tile([P, P], dtype=f32, tag="half")
                for k in range(cp):
                    s = mk_s(cum[b] + k)
                    nc.tensor.matmul(
                        out=psa[:, :m],
                        lhsT=xet[:, k * DIN:(k + 1) * DIN],
                        rhs=s[:, :m],
                        start=(k == 0), stop=(k == cp - 1))
                agg = sb.tile([P, P], dtype=bf16, tag="agg")
                nc.scalar.activation(out=agg[:, :m], in_=psa[:, :m],
                                     func=ACT.Copy)
                psb = [ph.tile([P, P], dtype=f32, tag="half", name=f"psb{h}")
                       for h in range(2)]
                for h in range(2):
                    nc.tensor.matmul(
                        out=psb[h][:, :m],
                        lhsT=w1sb[:, h * P:(h + 1) * P],
                        rhs=agg[:, :m],
                        start=True, stop=True)
                for h in range(2):
                    nc.scalar.activation(
                        out=xts[:, h * SH + b * P:h * SH + b * P + m],
                        in_=psb[h][:, :m],
                        func=ACT.Relu, bias=btsb[:, h:h + 1])

            def phase1(wsb, ybin, ybout):
                """Y = dinv * (h @ W) for own shard from xts; AllGather."""
                for w in range(NBLK):
                    m = LASTM if w == NBLK - 1 else P
                    ps = pp.tile([P, DH], dtype=f32, tag="ps")
                    for h in range(2):
                        nc.tensor.matmul(
                            out=ps[:m, :],
                            lhsT=xts[:, h * SH + w * P:h * SH + w * P + m],
                            rhs=wsb[:, h * DH:(h + 1) * DH],
                            start=(h == 0), stop=(h == 1))
                    ysb = sb.tile([P, DH], dtype=bf16, tag="ysb")
                    nc.scalar.activation(out=ysb[:m, :], in_=ps[:m, :],
                                         func=ACT.Copy,
                                         scale=dcosb[:m, w:w + 1])
                    nc.sync.dma_start(out=ybin[w * P:w * P + m, :],
                                      in_=ysb[:m, :])
                nc.gpsimd.collective_compute(
                    "AllGather", AG.bypass,
                    replica_groups=[list(range(NC))],
                    ins=[ybin.ap().opt()],
                    outs=[ybout.ap().opt()])

            def scatter_t(table, bofs):
                """Transposed scatter + fused bias/relu epilogue -> xts."""
                for b in range(NBLK):
                    cp = cpb[b]
                    m = LASTM if b == NBLK - 1 else P
                    gt = gp.tile([P, mxcp * DH], dtype=bf16, tag="gt")
                    for k in range(cp):
                        nc.gpsimd.indirect_dma_start(
                            out=gt[:, k * DH:(k + 1) * DH], out_offset=None,
                            in_=table[:, :],
                            in_offset=bass.IndirectOffsetOnAxis(
                                ap=idxsb[:, cum[b] + k:cum[b] + k + 1],
                                axis=0))
                    pst = [ph.tile([P, P], dtype=f32, tag="half", name=f"pst{h}")
                           for h in range(2)]
                    for k in range(cp):
                        s = mk_s(cum[b] + k)
                        for h in range(2):
                            nc.tensor.matmul(
                                out=pst[h][:, :m],
                                lhsT=gt[:, k * DH + h * P:k * DH + (h + 1) * P],
                                rhs=s[:, :m],
                                start=(k == 0), stop=(k == cp - 1))
                    for h in range(2):
                        nc.scalar.activation(
                            out=xts[:, h * SH + b * P:h * SH + b * P + m],
                            in_=pst[h][:, :m],
                            func=ACT.Relu, bias=btsb[:, bofs + h:bofs + h + 1])

            phase1(w2sb, ybin2, ybout2)
            scatter_t(ybout2, 2)
            phase1(w3sb, ybin3, ybout3)

            dbg = os.environ.get("KDBG")
            if dbg:
                # dump a bf16 [SH, DH] DRAM tensor to outp (cast to f32)
                src_t = {"yb2": ybin2, "yb3": ybin3}[dbg]
                for b in range(NBLK):
                    m = LASTM if b == NBLK - 1 else P
                    t = sb.tile([P, DH], dtype=bf16, tag="dbg")
                    nc.sync.dma_start(out=t[:m, :],
                                      in_=src_t[b * P:b * P + m, :])
                    t2 = sb.tile([P, DH], dtype=f32, tag="dbg2")
                    nc.vector.tensor_copy(out=t2[:m, :], in_=t[:m, :])
                    nc.sync.dma_start(out=outp[b * P:b * P + m, :],
                                      in_=t2[:m, :])

            # ---------------- Layer 3 scatter: direct [dst, feat] ----------
            for b in range(NBLK):
                cp = cpb[b]
                m = LASTM if b == NBLK - 1 else P
                gt = gp.tile([P, mxcp * DH], dtype=bf16, tag="gt")
                for k in range(cp):
                    nc.gpsimd.indirect_dma_start(
                        out=gt[:, k * DH:(k + 1) * DH], out_offset=None,
                        in_=ybout3[:, :],
                        in_offset=bass.IndirectOffsetOnAxis(
                            ap=idxsb[:, cum[b] + k:cum[b] + k + 1], axis=0))
                ps3 = pp.tile([P, DH], dtype=f32, tag="ps")
                for k in range(cp):
                    s = mk_s(cum[b] + k)
                    nc.tensor.matmul(
                        out=ps3[:m, :],
                        lhsT=s[:, :m],
                        rhs=gt[:, k * DH:(k + 1) * DH],
                        start=(k == 0), stop=(k == cp - 1))
                osb = sb.tile([P, DH], dtype=f32, tag="osb")
                nc.vector.tensor_tensor(out=osb[:m, :], in0=ps3[:m, :],
                                        in1=bf3sb[:m, :], op=AG.add)
                nc.sync.dma_start(out=outp[b * P:b * P + m, :],
                                  in_=osb[:m, :])

    nc.compile()
    return nc


def kernel(x, edge_index, W1, b1, W2, b2, W3, b3, _trace=False):
    from concourse.bass_utils import run_bass_kernel_spmd

    x = np.asarray(x, dtype=np.float32)
    per_core, meta = _preprocess(x, edge_index)
    nc = _build_program(meta)

    w2 = np.asarray(W2, np.float32)
    w3 = np.asarray(W3, np.float32)
    w2p = np.concatenate([w2[0:P, :], w2[P:2 * P, :]], axis=1).astype(BF16)
    w3p = np.concatenate([w3[0:P, :], w3[P:2 * P, :]], axis=1).astype(BF16)
    b1v = np.asarray(b1, np.float32)
    b2v = np.asarray(b2, np.float32)
    bt = np.stack([b1v[0:P], b1v[P:2 * P], b2v[0:P], b2v[P:2 * P]],
                  axis=1).astype(np.float32)
    jt = np.broadcast_to(np.arange(P, dtype=np.float32), (P, P))
    jt = np.ascontiguousarray(jt).astype(BF16)

    common = {
        "w1": np.asarray(W1, np.float32).astype(BF16),
        "w2p": w2p,
        "w3p": w3p,
        "jt": jt,
        "bt": bt,
        "bf3": np.broadcast_to(np.asarray(b3, np.float32), (P, DH)).copy(),
    }
    in_maps = []
    for c in range(NC):
        m = dict(common)
        m.update(per_core[c])
        m["dco"] = per_core[c]["dco"]
        in_maps.append(m)

    res = run_bass_kernel_spmd(nc, in_maps, list(range(NC)), trace=_trace)
    shards = [res.results[c]["out"] for c in range(NC)]
    out = np.concatenate(shards, axis=0)
    if _trace:
        return out, res
    return out
